# revision 11
# baseline (speedup 1.0000x reference)
"""Trainium2 Bass kernel for nn_KVEmbedding (embedding_lookup).

reference: out[b, l, :] = table[indices[b, l], :]
  indices: (4096, 200) int in [0, 1M); table: (1M, 64) f32
  out: (4096, 200, 64) f32

Strategy (8 NeuronCores): data-parallel over the batch dim — each core gets
512 of the 4096 index rows (102,400 lookups) and a full table replica in its
HBM. No collectives. Per core the output rows r = p*800 + g map to SBUF
partition p, free slot g; indirect DMAs gather 128 rows each ([128, 1] offset
AP = one offset per partition), staged through small SBUF tiles and written
back with contiguous descriptors.

HW findings driving this shape (validated by identity-table probes):
  - indirect_dma_start with a MULTI-offset AP ([128, k>1] or [1, N]) does NOT
    work on this hardware/ucode build: only the first offset per partition is
    honored, with the dst extent filled from contiguous table rows (a probe
    with consecutive row ids per partition masks this, so beware false
    positives). [1, N] offset APs with N*16B beyond the dynamic-DMA scratch
    crash the runtime outright.
  - The [128, 1] offset form (one row per partition per instruction) is
    correct and is the only usable gather shape, so the kernel issues 800
    such instructions per core. Per-instruction SWDGE descriptor generation
    (~1.04 us fixed, serial on the Pool engine) is then the wall: ~830 us.
  - Small staging chunks (4 gathers per writeback tile, 1-row drain tail)
    with a deep pool and a split index load shave the remaining pipeline
    stalls: ~838 us vs 869 us for 100-row chunks. Multi-offset forms were
    also re-probed as whole-tile offset APs and 3D dst APs — all broken.
"""

import numpy as np

N_CORES = 8
B, L = 4096, 200
V, D = 1_000_000, 64
P = 128
ROWS_PER_CORE = B * L // N_CORES  # 102400
G = ROWS_PER_CORE // P  # 800 lookups per partition
# 4 rows/partition per staging tile keeps the Pool engine streaming with
# minimal writeback stalls; the trailing 1-row chunks shorten the drain tail.
SCHEDULE = [4] * 198 + [1] * 8
BUFS = 24

_NC_CACHE: dict = {}


def build_nc(schedule=None, bufs=BUFS):
    from concourse import bass, mybir
    import concourse.bacc as bacc
    import concourse.tile as tile

    schedule = schedule or SCHEDULE
    assert sum(schedule) == G
    nc = bacc.Bacc(
        "TRN2", target_bir_lowering=False, debug=False, num_devices=N_CORES
    )
    table_t = nc.dram_tensor("table", [V, D], mybir.dt.float32, kind="ExternalInput")
    idx_t = nc.dram_tensor("idx", [P, G], mybir.dt.int32, kind="ExternalInput")
    out_t = nc.dram_tensor(
        "out", [ROWS_PER_CORE, D], mybir.dt.float32, kind="ExternalOutput"
    )

    with tile.TileContext(nc) as tc:
        with (
            tc.tile_pool(name="idxp", bufs=1) as ipool,
            tc.tile_pool(name="gath", bufs=bufs) as gpool,
        ):
            idx_sb = ipool.tile([P, G], mybir.dt.int32)
            iv = idx_t.ap()
            # split load: chunk 0's offsets land first so gathers start sooner
            c0 = schedule[0]
            nc.sync.dma_start(out=idx_sb[:, :c0], in_=iv[:, :c0])
            nc.sync.dma_start(out=idx_sb[:, c0:], in_=iv[:, c0:])

            out_view = out_t.ap().rearrange("(p g) d -> p g d", p=P)
            pos = 0
            for chunk in schedule:
                gt = gpool.tile([P, chunk * D], mybir.dt.float32, tag="gt")
                for g in range(chunk):
                    nc.gpsimd.indirect_dma_start(
                        out=gt[:, g * D : (g + 1) * D],
                        out_offset=None,
                        in_=table_t.ap(),
                        in_offset=bass.IndirectOffsetOnAxis(
                            ap=idx_sb[:, pos + g : pos + g + 1],
                            axis=0,
                        ),
                    )
                nc.sync.dma_start(
                    out=out_view[:, pos : pos + chunk, :], in_=gt[:]
                )
                pos += chunk

    nc.compile()
    return nc


def build_nc_fast(chunk=50, bufs=2):
    """Multi-offset form: 16 indirect DMAs of [128, chunk] offsets (~225 us
    model time). Broken on the current ucode build (one offset/partition
    honored); run_on_hw tries it, verifies a host-side sample, and falls back
    to the safe form. On a fixed ucode this becomes the selected kernel."""
    from concourse import bass, mybir
    import concourse.bacc as bacc
    import concourse.tile as tile

    nchunk = G // chunk
    nc = bacc.Bacc(
        "TRN2",
        target_bir_lowering=False,
        debug=False,
        num_devices=N_CORES,
        dynamic_dma_scratch_size=2**17,  # 8192-desc carveout >= 6400/inst
    )
    table_t = nc.dram_tensor("table", [V, D], mybir.dt.float32, kind="ExternalInput")
    idx_t = nc.dram_tensor("idx", [P, G], mybir.dt.int32, kind="ExternalInput")
    out_t = nc.dram_tensor(
        "out", [ROWS_PER_CORE, D], mybir.dt.float32, kind="ExternalOutput"
    )
    with tile.TileContext(nc) as tc:
        with (
            tc.tile_pool(name="idxp", bufs=1) as ipool,
            tc.tile_pool(name="gath", bufs=bufs) as gpool,
        ):
            idx_sb = ipool.tile([P, G], mybir.dt.int32)
            nc.sync.dma_start(out=idx_sb[:], in_=idx_t.ap())
            out_view = out_t.ap().rearrange("(p g) d -> p g d", p=P)
            for c in range(nchunk):
                gt = gpool.tile([P, chunk * D], mybir.dt.float32, tag="gt")
                nc.gpsimd.indirect_dma_start(
                    out=gt[:],
                    out_offset=None,
                    in_=table_t.ap(),
                    in_offset=bass.IndirectOffsetOnAxis(
                        ap=idx_sb[:, c * chunk : (c + 1) * chunk], axis=0
                    ),
                )
                nc.sync.dma_start(
                    out=out_view[:, c * chunk : (c + 1) * chunk, :], in_=gt[:]
                )
    nc.compile()
    return nc


def _get_nc():
    if "nc" not in _NC_CACHE:
        _NC_CACHE["nc"] = build_nc()
    return _NC_CACHE["nc"]


def _sample_ok(out_full: np.ndarray, indices: np.ndarray, table: np.ndarray) -> bool:
    """Exact spot-check of the gather on a host-side sample (no quantization
    anywhere, so matches must be bit-exact)."""
    flat_idx = np.ascontiguousarray(indices.astype(np.int64, copy=False)).reshape(-1)
    out2d = out_full.reshape(-1, D)
    rng = np.random.default_rng(0)
    pos = rng.integers(0, flat_idx.size, size=4096)
    return bool(np.array_equal(out2d[pos], table[flat_idx[pos]]))


def make_in_maps(indices: np.ndarray, table: np.ndarray) -> list[dict]:
    idx = np.ascontiguousarray(indices.astype(np.int32, copy=False)).reshape(
        N_CORES, P, G
    )  # [core, p, g] = flat[core, p*G + g]
    table = np.ascontiguousarray(np.asarray(table, dtype=np.float32))
    return [
        {"table": table, "idx": np.ascontiguousarray(idx[i])}
        for i in range(N_CORES)
    ]


def assemble_out(results: list[dict]) -> np.ndarray:
    outs = [results[i]["out"].reshape(B // N_CORES, L, D) for i in range(N_CORES)]
    return np.concatenate(outs, axis=0)


def run_on_hw(indices: np.ndarray, table: np.ndarray, **spmd_kwargs):
    from concourse.bass_utils import run_bass_kernel_spmd

    in_maps = make_in_maps(indices, table)
    table_f = in_maps[0]["table"]

    # Capability-adaptive: try the 16-instruction multi-offset kernel and
    # verify a sample of the real output on host. On ucode builds where
    # multi-offset indirect DMA works this selects the ~4x faster program;
    # on the current build the sample check fails and we fall back.
    if "nc" not in _NC_CACHE:
        try:
            nc_fast = build_nc_fast()
            res = run_bass_kernel_spmd(
                nc_fast, in_maps, core_ids=list(range(N_CORES)), **spmd_kwargs
            )
            out = assemble_out(res.results)
            if _sample_ok(out, indices, table_f):
                _NC_CACHE["nc"] = nc_fast
                return out, res
        except Exception:
            pass

    nc = _get_nc()
    res = run_bass_kernel_spmd(
        nc, in_maps, core_ids=list(range(N_CORES)), **spmd_kwargs
    )
    return assemble_out(res.results), res


def kernel(indices: np.ndarray, table: np.ndarray, dummy=None, **_unused) -> np.ndarray:
    out, _ = run_on_hw(np.asarray(indices), np.asarray(table))
    return out
